# revision 1
# baseline (speedup 1.0000x reference)
"""Trainium2 Bass kernel for CustomWavLMAttention (B=4, T=1024, E=768, H=12).

Sharding: 8 cores; core c handles batch b=c//2 and query-half th=c%2
(512 query tokens). Each core redundantly computes k/v for its full batch
(no collectives), q/attention/output projection for its 512 rows.

v4 highlights:
- Host folds the reference's double projection (+LoRA) into one affine map
  W_eff = (W^T + 0.5 A^T B^T) W^T, b_eff = b W^T + b (q absorbs 1/sqrt(hd)).
- Projection weights and activations stream in bf16 (halves DMA, same PE
  rate); accumulation stays fp32 in PSUM.
- HW runs partial-array matmuls (K<128 or M<=64 or partial rhs partitions)
  at ~half rate, so every broadcast/score matmul is padded to full 128:
  q lives in a per-head zero-padded [128, 2*512] layout (score contraction
  K=64 -> 128), the gate broadcast uses a 128-row selector, and the
  1/sum broadcast uses a delta(k=0) selector against a 128-row tile.
- v is token-major with a per-head interleaved ones column, so each head's
  ctx matmul emits the softmax denominator as PSUM row 64 for free.
- 1/sum uses the single-pass reciprocal_approx_fast (the exact reciprocal
  costs ~6 DVE passes).
- Relative-position table rb is host-computed, shipped bf16; the gated
  staircase multiply runs on DVE in bf16 (2x) and is folded into the score
  PSUM by a bf16 anti-diagonal matmul.
"""

from contextlib import ExitStack

import numpy as np

import concourse.bass as bass
import concourse.mybir as mybir
import concourse.tile as tile
from concourse import bacc
from concourse.bass_utils import run_bass_kernel_spmd

F32 = mybir.dt.float32
F32R = mybir.dt.float32r
BF16 = mybir.dt.bfloat16
AF = mybir.ActivationFunctionType
ALU = mybir.AluOpType

B, T, E, H, HD = 4, 1024, 768, 12, 64
KT = E // 128             # 6 feature tiles
TT = T // 128             # 8 token tiles
QW = 512                  # query tokens per core
VW = H * 65               # 780: v layout with per-head ones column
NB = 320                  # rel buckets
RBW = 1664                # rb table width (>= 1535)
SW = 1408                 # staircase width
N_CORES = 8


def _bucket1d():
    """bucket index for rel = j - i, rel in [-1023, 1023] (idx = rel + 1023).

    numpy replica of reference._rel_bucket (f32 math, trunc-toward-zero)."""
    rel = np.arange(-1023, 1024)
    nb = NB // 2                                   # 160
    buckets = (rel > 0).astype(np.int64) * nb
    arel = np.abs(rel)
    max_exact = nb // 2                            # 80
    is_small = arel < max_exact
    log_ratio = np.log(np.maximum(arel, 1).astype(np.float32)
                       / np.float32(max_exact))
    large = max_exact + (
        log_ratio / np.float32(np.log(800.0 / max_exact))
        * np.float32(nb - max_exact)
    ).astype(np.int32)
    large = np.minimum(large, nb - 1)
    return (buckets + np.where(is_small, arel, large)).astype(np.int64)


def _build_program():
    nc = bacc.Bacc("TRN2", target_bir_lowering=False)

    def inp(name, shape, dt):
        return nc.dram_tensor(name, shape, dt, kind="ExternalInput")

    xT = inp("xT", [E, T], BF16)        # batch's hidden, transposed
    xq = inp("xq", [E, QW], BF16)       # this core's query half of xT
    wq_e = inp("wq_e", [E, E], BF16)    # folded q weight (f_in, e_out)
    wk_e = inp("wk_e", [E, E], BF16)
    wv_a = inp("wv_a", [E, VW], BF16)   # folded v weight, ones-col layout
    wo_t = inp("wo_t", [E, E], BF16)
    bq_c = inp("bq_c", [E, 1], F32)
    bk_c = inp("bk_c", [E, 1], F32)
    bo_c = inp("bo_c", [E, 1], F32)
    bv_rep = inp("bv_rep", [128, VW], BF16)   # bv_eff row replicated (+ones)
    wg_big = inp("wg_big", [E, 64], BF16)
    bg_c = inp("bg_c", [64, 1], F32)
    anti = inp("anti", [128, 128], BF16)
    e0_pad = inp("e0_pad", [128, 128], F32R)  # delta(k==0) broadcaster
    sel_pad = inp("sel_pad", [128, H * 128], F32R)
    rbrev = inp("rbrev", [H, RBW], BF16)

    outT = nc.dram_tensor("outT", [E, QW], F32, kind="ExternalOutput")

    with tile.TileContext(nc) as tc:
        with ExitStack() as es:
            consts = es.enter_context(tc.tile_pool(name="consts", bufs=1))
            persist = es.enter_context(tc.tile_pool(name="persist", bufs=1))

            anti_sb = consts.tile([128, 128], BF16, tag="anti", name="anti")
            nc.sync.dma_start(out=anti_sb, in_=anti[:, :])
            e0_sb = consts.tile([128, 128], F32R, tag="e0", name="e0")
            nc.sync.dma_start(out=e0_sb, in_=e0_pad[:, :])
            bvr_sb = consts.tile([128, VW], BF16, tag="bvr", name="bvr")
            nc.sync.dma_start(out=bvr_sb, in_=bv_rep[:, :])
            bg_sb = consts.tile([64, 1], F32, tag="bg", name="bg")
            nc.sync.dma_start(out=bg_sb, in_=bg_c[:, :])
            sel_sb = consts.tile([128, H * 128], F32R, tag="sel", name="sel")
            nc.sync.dma_start(out=sel_sb, in_=sel_pad[:, :])
            bias_cols = {}
            for nm, src in (("q", bq_c), ("k", bk_c), ("o", bo_c)):
                t = consts.tile([128, KT], F32, tag=f"b{nm}c", name=f"b{nm}c")
                nc.sync.dma_start(out=t, in_=bass.AP(
                    tensor=src[:, :].tensor, offset=0, ap=[[1, 128], [128, KT]]))
                bias_cols[nm] = t

            # persistent activations
            gfin_sb = persist.tile([128, QW], F32R, tag="gfin", name="gfin")
            qTz_sb = [persist.tile([128, 2 * QW], F32R, tag=f"qz{i}",
                                   name=f"qz{i}") for i in range(KT)]
            kT_sb = [persist.tile([128, T], F32R, tag=f"kT{i}", name=f"kT{i}")
                     for i in range(KT)]
            vTok_sb = [persist.tile([128, VW], F32R, tag=f"vT{i}", name=f"vT{i}")
                       for i in range(TT)]
            ctx_sb = [persist.tile([128, QW], BF16, tag=f"ctx{i}", name=f"ctx{i}")
                      for i in range(KT)]
            # zero the pad halves of qTz (even head block: rows 64:128 of
            # cols 0:512; odd head block: rows 0:64 of cols 512:1024)
            for i in range(KT):
                nc.gpsimd.memset(qTz_sb[i][64:128, 0:QW].bitcast(F32), 0.0)
                nc.gpsimd.memset(qTz_sb[i][0:64, QW:2 * QW].bitcast(F32), 0.0)

            # ---------------- projections ----------------
            with ExitStack() as esP:
                wpool = esP.enter_context(tc.tile_pool(name="w", bufs=1))
                ps = esP.enter_context(
                    tc.tile_pool(name="ps", bufs=3, space="PSUM"))

                wg_sb = [wpool.tile([128, 64], BF16, tag=f"wg{i}", name=f"wg{i}")
                         for i in range(KT)]
                xq_sb = [wpool.tile([128, QW], BF16, tag=f"xq{i}", name=f"xq{i}")
                         for i in range(KT)]
                wq_sb = [wpool.tile([128, E], BF16, tag=f"wq{i}", name=f"wq{i}")
                         for i in range(KT)]
                wk_sb = [wpool.tile([128, E], BF16, tag=f"wk{i}", name=f"wk{i}")
                         for i in range(KT)]
                wv_sb = [wpool.tile([128, VW], BF16, tag=f"wv{i}", name=f"wv{i}")
                         for i in range(KT)]
                x_sb = [wpool.tile([128, T], BF16, tag=f"x{i}", name=f"x{i}")
                        for i in range(KT)]
                for i in range(KT):
                    r = slice(i * 128, (i + 1) * 128)
                    nc.sync.dma_start(out=wg_sb[i], in_=wg_big[r, :])
                    nc.sync.dma_start(out=xq_sb[i], in_=xq[r, :])
                    nc.sync.dma_start(out=wq_sb[i], in_=wq_e[r, :])
                    nc.sync.dma_start(out=wk_sb[i], in_=wk_e[r, :])
                    nc.sync.dma_start(out=wv_sb[i], in_=wv_a[r, :])
                    nc.sync.dma_start(out=x_sb[i], in_=xT[r, :])

                # gates: rows 0..11 = ga-logits, 32..43 = gb-logits
                psg = ps.tile([64, QW], F32, tag="ps", name="ps")
                for i in range(KT):
                    nc.tensor.matmul(psg, wg_sb[i], xq_sb[i],
                                     start=(i == 0), stop=(i == KT - 1))
                gsig_a = wpool.tile([H, QW], F32, tag="gsig_a", name="gsig_a")
                gsig_b = wpool.tile([H, QW], F32, tag="gsig_b", name="gsig_b")
                nc.scalar.activation(gsig_a, psg[0:H, :], AF.Sigmoid,
                                     bias=bg_sb[0:H, :])
                nc.scalar.activation(gsig_b, psg[32:32 + H, :], AF.Sigmoid,
                                     bias=bg_sb[32:32 + H, :])
                gprod = wpool.tile([H, QW], F32, tag="gprod", name="gprod")
                nc.vector.tensor_tensor(out=gprod, in0=gsig_a,
                                        in1=gsig_b, op=ALU.mult)
                # gate = ga*gb - ga + 2 = (prod + 2) - ga
                nc.vector.scalar_tensor_tensor(
                    out=gfin_sb[0:H, :], in0=gprod, scalar=2.0, in1=gsig_a,
                    op0=ALU.add, op1=ALU.subtract)

                # q projection -> zero-padded per-head-parity layout
                for i_o in range(KT):
                    c_o = slice(i_o * 128, (i_o + 1) * 128)
                    p = ps.tile([128, QW], F32, tag="ps", name="ps")
                    for i in range(KT):
                        nc.tensor.matmul(p, wq_sb[i][:, c_o], xq_sb[i],
                                         start=(i == 0), stop=(i == KT - 1))
                    nc.vector.tensor_scalar_add(
                        qTz_sb[i_o][0:64, 0:QW], p[0:64, :],
                        bias_cols["q"][0:64, i_o:i_o + 1])
                    nc.vector.tensor_scalar_add(
                        qTz_sb[i_o][64:128, QW:2 * QW], p[64:128, :],
                        bias_cols["q"][64:128, i_o:i_o + 1])
                # k projection over full T
                for i_o in range(KT):
                    c_o = slice(i_o * 128, (i_o + 1) * 128)
                    for ch in range(T // 512):
                        cs = slice(ch * 512, (ch + 1) * 512)
                        p = ps.tile([128, QW], F32, tag="ps", name="ps")
                        for i in range(KT):
                            nc.tensor.matmul(p, wk_sb[i][:, c_o],
                                             x_sb[i][:, cs],
                                             start=(i == 0), stop=(i == KT - 1))
                        nc.vector.tensor_scalar_add(
                            kT_sb[i_o][:, cs], p, bias_cols["k"][:, i_o:i_o + 1])
                # v projection, token-major, ones-col layout; bias via DVE add
                for tt in range(TT):
                    ts_ = slice(tt * 128, (tt + 1) * 128)
                    for ch, cw in ((0, 512), (1, VW - 512)):
                        cs = slice(ch * 512, ch * 512 + cw)
                        p = ps.tile([128, QW], F32, tag="ps", name="ps")
                        for i in range(KT):
                            nc.tensor.matmul(p[:, :cw], x_sb[i][:, ts_],
                                             wv_sb[i][:, cs],
                                             start=(i == 0), stop=(i == KT - 1))
                        nc.vector.tensor_tensor(out=vTok_sb[tt][:, cs],
                                                in0=p[:, :cw],
                                                in1=bvr_sb[:, cs], op=ALU.add)

            # ---------------- attention ----------------
            with ExitStack() as esC:
                stairp = esC.enter_context(tc.tile_pool(name="stair", bufs=2))
                wop = esC.enter_context(tc.tile_pool(name="wo", bufs=1))
                gatep = esC.enter_context(tc.tile_pool(name="gate", bufs=2))
                gp = esC.enter_context(tc.tile_pool(name="G", bufs=4))
                expp = esC.enter_context(tc.tile_pool(name="expt", bufs=8))
                recp = esC.enter_context(tc.tile_pool(name="recp", bufs=2))
                smallp = esC.enter_context(tc.tile_pool(name="small", bufs=2))
                ps_sc = esC.enter_context(
                    tc.tile_pool(name="ps_sc", bufs=2, space="PSUM"))
                ps_ctx = esC.enter_context(
                    tc.tile_pool(name="ps_ctx", bufs=2, space="PSUM"))
                ps_g = esC.enter_context(
                    tc.tile_pool(name="ps_g", bufs=2, space="PSUM"))

                # pre-zeroed reciprocal-row tiles (rows 1.. stay 0; the PE
                # broadcast multiplies them by e0's zero rows)
                rec_tiles = [recp.tile([128, QW], F32R, tag=f"recr{i}",
                                       name=f"recr{i}") for i in range(2)]
                for rt in rec_tiles:
                    nc.gpsimd.memset(rt[:, :].bitcast(F32), 0.0)

                # prefetch the first two staircases before the wo weights
                stair_tiles = {}
                for h in range(2):
                    st = stairp.tile([128, SW], BF16, tag="stair", name="stair")
                    nc.sync.dma_start(out=st, in_=bass.AP(
                        tensor=rbrev[:, :].tensor,
                        offset=h * RBW, ap=[[1, 128], [1, SW]]))
                    stair_tiles[h] = st

                wo_sb = [wop.tile([128, E], BF16, tag=f"wo{i}", name=f"wo{i}")
                         for i in range(KT)]
                for i in range(KT):
                    nc.sync.dma_start(out=wo_sb[i],
                                      in_=wo_t[i * 128:(i + 1) * 128, :])

                for h in range(H):
                    kt, half = h // 2, (h % 2) * 64
                    if h in stair_tiles:
                        stair = stair_tiles[h]
                    else:
                        stair = stairp.tile([128, SW], BF16, tag="stair",
                                            name="stair")
                        nc.sync.dma_start(out=stair, in_=bass.AP(
                            tensor=rbrev[:, :].tensor,
                            offset=h * RBW, ap=[[1, 128], [1, SW]]))
                    pg = ps_g.tile([128, QW], F32, tag="pg", name="pg")
                    nc.tensor.matmul(
                        pg, sel_sb[:, h * 128:(h + 1) * 128],
                        gfin_sb, start=True, stop=True)
                    gate_bc = gatep.tile([128, QW], BF16, tag="gbc", name="gbc")
                    nc.vector.tensor_copy(gate_bc, pg)
                    ps_c = ps_ctx.tile([65, QW], F32, tag="psc", name="psc")
                    for jt in range(TT):
                        ms = 896 - jt * 128
                        G = gp.tile([128, QW], BF16, tag="G", name="G")
                        nc.vector.tensor_tensor(out=G, in0=stair[:, ms:ms + QW],
                                                in1=gate_bc, op=ALU.mult)
                        pss = ps_sc.tile([128, QW], F32, tag="pss", name="pss")
                        nc.tensor.matmul(
                            pss, kT_sb[kt][:, jt * 128:(jt + 1) * 128],
                            qTz_sb[kt][:, (h % 2) * QW:(h % 2) * QW + QW],
                            start=True, stop=False)
                        nc.tensor.matmul(pss, anti_sb, G, start=False,
                                         stop=True)
                        expT = expp.tile([128, QW], F32R, tag="expt",
                                         name="expt")
                        nc.scalar.activation(expT, pss, AF.Exp)
                        nc.tensor.matmul(ps_c,
                                         vTok_sb[jt][:, h * 65:h * 65 + 65],
                                         expT, start=(jt == 0),
                                         stop=(jt == TT - 1))
                    # 1/sum as exp(-ln(sum)) on the Act engine (the DVE
                    # reciprocal is a ~6-pass op; approx_fast can't read PSUM)
                    lns = recp.tile([1, QW], F32, tag="lns", name="lns")
                    nc.scalar.activation(lns, ps_c[64:65, :], AF.Ln)
                    rr = rec_tiles[h % 2]
                    nc.scalar.activation(rr[0:1, :], lns, AF.Exp, scale=-1.0)
                    pr = ps_g.tile([128, QW], F32, tag="pg", name="pg")
                    nc.tensor.matmul(pr, e0_sb, rr, start=True, stop=True)
                    rec_sb = smallp.tile([64, QW], F32, tag="recsb",
                                         name="recsb")
                    nc.vector.tensor_copy(rec_sb, pr[0:64, :])
                    nc.vector.tensor_tensor(out=ctx_sb[kt][half:half + 64, :],
                                            in0=ps_c[0:64, :], in1=rec_sb,
                                            op=ALU.mult)

                # ---------------- output projection ----------------
                for i_o in range(KT):
                    c_o = slice(i_o * 128, (i_o + 1) * 128)
                    p = ps_sc.tile([128, QW], F32, tag="pss", name="pss")
                    for i in range(KT):
                        nc.tensor.matmul(p, wo_sb[i][:, c_o], ctx_sb[i],
                                         start=(i == 0), stop=(i == KT - 1))
                    o_sb = smallp.tile([128, QW], F32, tag="osb", name="osb")
                    nc.vector.tensor_scalar_add(o_sb, p,
                                                bias_cols["o"][:, i_o:i_o + 1])
                    nc.sync.dma_start(out=outT[c_o, :], in_=o_sb)

    nc.finalize()
    return nc


_NC_CACHE = None


def _get_nc():
    global _NC_CACHE
    if _NC_CACHE is None:
        _NC_CACHE = _build_program()
    return _NC_CACHE


def kernel(hidden_states, Wq, bq, Wk, bk, Wv, bv,
           Aq, Bq, Ak, Bk, Av, Bv, Wo, bo, Wg, bg, gru_const, rel_embed):
    import ml_dtypes

    BF = ml_dtypes.bfloat16
    hidden_states = np.asarray(hidden_states, dtype=np.float32)
    f = lambda a: np.ascontiguousarray(np.asarray(a, dtype=np.float32))

    # ---- fold the double projection (+LoRA) into one affine map ----
    def fold(W, b, A, Bm, scale=1.0):
        W, b, A, Bm = f(W), f(b), f(A), f(Bm)
        M = (W.T + 0.5 * (A.T @ Bm.T)) @ W.T * scale
        be = (b @ W.T + b) * scale
        return M, be

    Mq, bq_e = fold(Wq, bq, Aq, Bq, float(HD) ** -0.5)
    Mk, bk_e = fold(Wk, bk, Ak, Bk)
    Mv, bv_e = fold(Wv, bv, Av, Bv)

    wv_a = np.zeros((E, VW), np.float32)
    bv_row = np.zeros(VW, np.float32)
    for h in range(H):
        wv_a[:, h * 65:h * 65 + 64] = Mv[:, h * 64:(h + 1) * 64]
        bv_row[h * 65:h * 65 + 64] = bv_e[h * 64:(h + 1) * 64]
        bv_row[h * 65 + 64] = 1.0
    bv_rep = np.broadcast_to(bv_row, (128, VW))

    shared = {
        "wq_e": Mq.astype(BF), "wk_e": Mk.astype(BF),
        "wv_a": wv_a.astype(BF), "wo_t": f(Wo.T).astype(BF),
        "bq_c": bq_e.reshape(E, 1).astype(np.float32),
        "bk_c": bk_e.reshape(E, 1).astype(np.float32),
        "bo_c": f(bo).reshape(E, 1),
        "bv_rep": np.ascontiguousarray(bv_rep.astype(BF)),
    }
    anti = np.zeros((128, 128), np.float32)
    anti[np.arange(128), 127 - np.arange(128)] = 1.0
    shared["anti"] = anti.astype(BF)
    e0 = np.zeros((128, 128), np.float32)
    e0[0, :] = 1.0
    shared["e0_pad"] = e0
    sel = np.zeros((128, H * 128), np.float32)
    for h in range(H):
        sel[h, h * 128:(h + 1) * 128] = 1.0
    shared["sel_pad"] = sel
    # gate projection: fold the reshape(2,4).sum(-1) into the weights and lay
    # out block-diagonally per head. gru_const == 1 is folded into the gate
    # algebra (gate = ga*gb - ga + 2).
    Wg_np, bg_np = f(Wg), f(bg)
    wg2 = Wg_np.reshape(2, 4, HD).sum(1)            # [2, HD]
    bg2 = bg_np.reshape(2, 4).sum(1)                # [2]
    wg_big = np.zeros((E, 64), np.float32)
    for h in range(H):
        wg_big[h * HD:(h + 1) * HD, h] = wg2[0]
        wg_big[h * HD:(h + 1) * HD, 32 + h] = wg2[1]
    shared["wg_big"] = wg_big.astype(BF)
    bg_c = np.zeros((64, 1), np.float32)
    bg_c[:H, 0] = bg2[0]
    bg_c[32:32 + H, 0] = bg2[1]
    shared["bg_c"] = bg_c

    # host-computed reversed rb table:
    # rbrev[h, u] = rel_embed[b1d[2046 - th*512 - u], h] (0 where invalid)
    b1d = _bucket1d()
    rel = f(rel_embed)
    rbrev = {}
    for th in range(2):
        m = np.zeros((H, RBW), np.float32)
        u = np.arange(RBW)
        src = 2046 - th * QW - u
        ok = (src >= 0) & (src <= 2046)
        m[:, u[ok]] = rel[b1d[src[ok]], :].T
        rbrev[th] = m.astype(BF)

    xT_all = hidden_states.transpose(0, 2, 1).astype(BF)  # [B,E,T] bf16

    in_maps = []
    for c in range(N_CORES):
        b_, th = c // 2, c % 2
        im = dict(shared)
        im["xT"] = np.ascontiguousarray(xT_all[b_])
        im["xq"] = np.ascontiguousarray(xT_all[b_][:, th * QW:(th + 1) * QW])
        im["rbrev"] = rbrev[th]
        in_maps.append(im)

    nc = _get_nc()
    res = run_bass_kernel_spmd(nc, in_maps, core_ids=list(range(N_CORES)))

    out = np.empty((B, T, E), np.float32)
    for c in range(N_CORES):
        b_, th = c // 2, c % 2
        out[b_, th * QW:(th + 1) * QW, :] = res.results[c]["outT"].T
    return out



# revision 8
# speedup vs baseline: 1.7185x; 1.7185x over previous
"""Trainium2 Bass kernel for CustomWavLMAttention (B=4, T=1024, E=768, H=12).

Sharding: 8 cores; core c handles batch b=c//2 and query-half th=c%2
(512 query tokens). Each core redundantly computes k/v for its full batch
(no collectives), q/attention/output projection for its 512 rows.

v5 highlights (vs v4 baseline at 293us):
- k bias dropped (adds a per-query constant to all logits -> softmax
  invariant); v bias folded into bo (softmax rows sum to 1). Fewer
  DVE bias passes, fewer inputs.
- Softmax 1/sum on DVE via reciprocal_approx_fast (v4 used Ln+Exp on ACT,
  which thrashed the activation table set twice per head: 24 extra
  ACT_TABLE_LOADs = ~31us of Scalar-engine time and a 6us serial bubble
  per head that let the PE HAM throttle to half clock for 125us).
- Normalization chain (sum copy -> reciprocal -> PE broadcast -> ctx mult)
  is software-pipelined one head late, ctx matmuls one jt-pair late, so
  the PE queue never waits on DVE/ACT results.
- All 12 gate broadcasts precomputed in the projection phase.
- exp batched over two PSUM banks ([128,1024] per call: 48 calls instead
  of 96 halves the ACT fixed overhead).
- q/k/v/expT/ctx all bf16 (PE full rate, half LDWEIGHTS time, half SBUF).
- DMAs issued in compute-priority order (gates/q weights first, wo last)
  so projections start at ~5us instead of ~35us.
"""

from contextlib import ExitStack

import numpy as np

import concourse.bass as bass
import concourse.mybir as mybir
import concourse.tile as tile
from concourse import bacc
from concourse.bass_utils import run_bass_kernel_spmd

F32 = mybir.dt.float32
F32R = mybir.dt.float32r
BF16 = mybir.dt.bfloat16
AF = mybir.ActivationFunctionType
ALU = mybir.AluOpType

B, T, E, H, HD = 4, 1024, 768, 12, 64
KT = E // 128             # 6 feature tiles
TT = T // 128             # 8 token tiles
QW = 512                  # query tokens per core
VW = H * 65               # 780: v layout with per-head ones column
NB = 320                  # rel buckets
RBW = 1664                # rb table width (>= 1535)
SW = 1408                 # staircase width
N_CORES = 8


def _bucket1d():
    """bucket index for rel = j - i, rel in [-1023, 1023] (idx = rel + 1023).

    numpy replica of reference._rel_bucket (f32 math, trunc-toward-zero)."""
    rel = np.arange(-1023, 1024)
    nb = NB // 2                                   # 160
    buckets = (rel > 0).astype(np.int64) * nb
    arel = np.abs(rel)
    max_exact = nb // 2                            # 80
    is_small = arel < max_exact
    log_ratio = np.log(np.maximum(arel, 1).astype(np.float32)
                       / np.float32(max_exact))
    large = max_exact + (
        log_ratio / np.float32(np.log(800.0 / max_exact))
        * np.float32(nb - max_exact)
    ).astype(np.int32)
    large = np.minimum(large, nb - 1)
    return (buckets + np.where(is_small, arel, large)).astype(np.int64)


def _build_program():
    nc = bacc.Bacc("TRN2", target_bir_lowering=False)

    def inp(name, shape, dt):
        return nc.dram_tensor(name, shape, dt, kind="ExternalInput")

    xT = inp("xT", [E, T], BF16)        # batch's hidden, transposed
    xq = inp("xq", [E, QW], BF16)       # this core's query half of xT
    wq_e = inp("wq_e", [E, E], BF16)    # folded q weight (f_in, e_out)
    wk_e = inp("wk_e", [E, E], BF16)
    wv_a = inp("wv_a", [E, VW], BF16)   # folded v weight, ones-col layout
    wo_t = inp("wo_t", [E, E], BF16)
    bq_c = inp("bq_c", [E, 1], F32)
    bo_c = inp("bo_c", [E, 1], F32)     # bo + Wo @ bv (bv folded here)
    ones_rep = inp("ones_rep", [128, VW], BF16)  # ones-col indicator rows
    wg_big = inp("wg_big", [E, 64], BF16)
    bg_c = inp("bg_c", [64, 1], F32)
    anti = inp("anti", [128, 128], BF16)
    e0_pad = inp("e0_pad", [128, 128], F32R)  # delta(k==0) broadcaster
    sel_pad = inp("sel_pad", [128, H * 128], BF16)
    rbrev = inp("rbrev", [H, RBW], BF16)

    outT = nc.dram_tensor("outT", [E, QW], F32, kind="ExternalOutput")

    with tile.TileContext(nc) as tc:
        with ExitStack() as es:
            consts = es.enter_context(tc.tile_pool(name="consts", bufs=1))
            persist = es.enter_context(tc.tile_pool(name="persist", bufs=1))

            # ---- small consts first (cheap issues before the big weights)
            bg_sb = consts.tile([64, 1], F32, tag="bg", name="bg")
            nc.sync.dma_start(out=bg_sb, in_=bg_c[:, :])
            bias_cols = {}
            for nm, src in (("q", bq_c), ("o", bo_c)):
                t = consts.tile([128, KT], F32, tag=f"b{nm}c", name=f"b{nm}c")
                nc.sync.dma_start(out=t, in_=bass.AP(
                    tensor=src[:, :].tensor, offset=0, ap=[[1, 128], [128, KT]]))
                bias_cols[nm] = t
            anti_sb = consts.tile([128, 128], BF16, tag="anti", name="anti")
            nc.sync.dma_start(out=anti_sb, in_=anti[:, :])
            e0_sb = consts.tile([128, 128], F32R, tag="e0", name="e0")
            nc.sync.dma_start(out=e0_sb, in_=e0_pad[:, :])
            sel_sb = consts.tile([128, H * 128], BF16, tag="sel", name="sel")
            nc.sync.dma_start(out=sel_sb, in_=sel_pad[:, :])
            onesr_sb = consts.tile([128, VW], BF16, tag="onesr", name="onesr")
            nc.sync.dma_start(out=onesr_sb, in_=ones_rep[:, :])

            # persistent activations
            gfin_sb = persist.tile([128, QW], BF16, tag="gfin", name="gfin")
            qTz_sb = [persist.tile([128, 2 * QW], BF16, tag=f"qz{i}",
                                   name=f"qz{i}") for i in range(KT)]
            kT_sb = [persist.tile([128, T], BF16, tag=f"kT{i}", name=f"kT{i}")
                     for i in range(KT)]
            vTok_sb = [persist.tile([128, VW], BF16, tag=f"vT{i}", name=f"vT{i}")
                       for i in range(TT)]
            ctx_sb = [persist.tile([128, QW], BF16, tag=f"ctx{i}", name=f"ctx{i}")
                      for i in range(KT)]
            gate_bc = [persist.tile([128, QW], BF16, tag=f"gbc{i}",
                                    name=f"gbc{i}") for i in range(H)]
            hctx = [persist.tile([64, QW], BF16, tag=f"hctx{i}",
                                 name=f"hctx{i}") for i in range(H)]
            sums_sb = persist.tile([1, QW], F32, tag="sums", name="sums")
            recf_sb = persist.tile([1, QW], F32, tag="recf", name="recf")
            rec_sb = persist.tile([128, QW], F32R, tag="rec", name="rec")

            # zero: q pad halves (even head block: rows 64:128 of cols 0:512;
            # odd head block: rows 0:64 of cols 512:1024); gfin garbage rows
            # (sel rows >=12 are 0 but 0*NaN would poison PSUM); rec rows 1+.
            for i in range(KT):
                nc.gpsimd.memset(qTz_sb[i][64:128, 0:QW], 0.0)
                nc.gpsimd.memset(qTz_sb[i][0:64, QW:2 * QW], 0.0)
            nc.gpsimd.memset(gfin_sb, 0.0)
            nc.gpsimd.memset(rec_sb.bitcast(F32), 0.0)

            # ---------------- projections ----------------
            with ExitStack() as esP:
                wpool = esP.enter_context(tc.tile_pool(name="w", bufs=1))
                ps = esP.enter_context(
                    tc.tile_pool(name="ps", bufs=3, space="PSUM"))

                # DMA issue order = compute priority order:
                # wg+xq (gates), wq (q proj), xT+wk (k proj), wv, wo last.
                wg_sb = [wpool.tile([128, 64], BF16, tag=f"wg{i}", name=f"wg{i}")
                         for i in range(KT)]
                xq_sb = [wpool.tile([128, QW], BF16, tag=f"xq{i}", name=f"xq{i}")
                         for i in range(KT)]
                wq_sb = [wpool.tile([128, E], BF16, tag=f"wq{i}", name=f"wq{i}")
                         for i in range(KT)]
                wk_sb = [wpool.tile([128, E], BF16, tag=f"wk{i}", name=f"wk{i}")
                         for i in range(KT)]
                wv_sb = [wpool.tile([128, VW], BF16, tag=f"wv{i}", name=f"wv{i}")
                         for i in range(KT)]
                x_sb = [wpool.tile([128, T], BF16, tag=f"x{i}", name=f"x{i}")
                        for i in range(KT)]
                for i in range(KT):
                    r = slice(i * 128, (i + 1) * 128)
                    nc.sync.dma_start(out=wg_sb[i], in_=wg_big[r, :])
                    nc.sync.dma_start(out=xq_sb[i], in_=xq[r, :])
                for i in range(KT):
                    r = slice(i * 128, (i + 1) * 128)
                    nc.sync.dma_start(out=wq_sb[i], in_=wq_e[r, :])
                for i in range(KT):
                    r = slice(i * 128, (i + 1) * 128)
                    nc.sync.dma_start(out=x_sb[i], in_=xT[r, :])
                    nc.sync.dma_start(out=wk_sb[i], in_=wk_e[r, :])
                for i in range(KT):
                    r = slice(i * 128, (i + 1) * 128)
                    nc.sync.dma_start(out=wv_sb[i], in_=wv_a[r, :])

                # gates: rows 0..11 = ga-logits, 32..43 = gb-logits
                psg = ps.tile([64, QW], F32, tag="ps", name="ps")
                for i in range(KT):
                    nc.tensor.matmul(psg, wg_sb[i], xq_sb[i],
                                     start=(i == 0), stop=(i == KT - 1))
                gsig_a = wpool.tile([H, QW], F32, tag="gsig_a", name="gsig_a")
                gsig_b = wpool.tile([H, QW], F32, tag="gsig_b", name="gsig_b")
                nc.scalar.activation(gsig_a, psg[0:H, :], AF.Sigmoid,
                                     bias=bg_sb[0:H, :])
                nc.scalar.activation(gsig_b, psg[32:32 + H, :], AF.Sigmoid,
                                     bias=bg_sb[32:32 + H, :])
                gprod = wpool.tile([H, QW], F32, tag="gprod", name="gprod")
                nc.vector.tensor_tensor(out=gprod, in0=gsig_a,
                                        in1=gsig_b, op=ALU.mult)
                # gate = ga*gb - ga + 2 = (prod + 2) - ga
                nc.vector.scalar_tensor_tensor(
                    out=gfin_sb[0:H, :], in0=gprod, scalar=2.0, in1=gsig_a,
                    op0=ALU.add, op1=ALU.subtract)

                # broadcast all 12 head gates to [128, QW] bf16 tiles now;
                # the DVE casts overlap the q/k/v projection matmuls.
                for h in range(H):
                    pg = ps.tile([128, QW], F32, tag="ps", name="ps")
                    nc.tensor.matmul(pg, sel_sb[:, h * 128:(h + 1) * 128],
                                     gfin_sb, start=True, stop=True)
                    nc.vector.tensor_copy(gate_bc[h], pg)

                # q projection -> zero-padded per-head-parity layout
                for i_o in range(KT):
                    c_o = slice(i_o * 128, (i_o + 1) * 128)
                    p = ps.tile([128, QW], F32, tag="ps", name="ps")
                    for i in range(KT):
                        nc.tensor.matmul(p, wq_sb[i][:, c_o], xq_sb[i],
                                         start=(i == 0), stop=(i == KT - 1))
                    nc.vector.tensor_scalar_add(
                        qTz_sb[i_o][0:64, 0:QW], p[0:64, :],
                        bias_cols["q"][0:64, i_o:i_o + 1])
                    nc.vector.tensor_scalar_add(
                        qTz_sb[i_o][64:128, QW:2 * QW], p[64:128, :],
                        bias_cols["q"][64:128, i_o:i_o + 1])
                # k projection over full T (no bias: constant per query row,
                # softmax-invariant)
                for i_o in range(KT):
                    c_o = slice(i_o * 128, (i_o + 1) * 128)
                    for ch in range(T // 512):
                        cs = slice(ch * 512, (ch + 1) * 512)
                        p = ps.tile([128, QW], F32, tag="ps", name="ps")
                        for i in range(KT):
                            nc.tensor.matmul(p, wk_sb[i][:, c_o],
                                             x_sb[i][:, cs],
                                             start=(i == 0), stop=(i == KT - 1))
                        nc.vector.tensor_copy(kT_sb[i_o][:, cs], p)
                # v projection, token-major, ones-col layout (bv folded
                # into bo on host; the add just plants the ones columns)
                for tt in range(TT):
                    ts_ = slice(tt * 128, (tt + 1) * 128)
                    for ch, cw in ((0, 512), (1, VW - 512)):
                        cs = slice(ch * 512, ch * 512 + cw)
                        p = ps.tile([128, QW], F32, tag="ps", name="ps")
                        for i in range(KT):
                            nc.tensor.matmul(p[:, :cw], x_sb[i][:, ts_],
                                             wv_sb[i][:, cs],
                                             start=(i == 0), stop=(i == KT - 1))
                        nc.vector.tensor_tensor(out=vTok_sb[tt][:, cs],
                                                in0=p[:, :cw],
                                                in1=onesr_sb[:, cs], op=ALU.add)

            # ---------------- attention ----------------
            with ExitStack() as esC:
                stairp = esC.enter_context(tc.tile_pool(name="stair", bufs=3))
                wop = esC.enter_context(tc.tile_pool(name="wo", bufs=1))
                gp = esC.enter_context(tc.tile_pool(name="G", bufs=4))
                expp = esC.enter_context(tc.tile_pool(name="expt", bufs=4))
                smallp = esC.enter_context(tc.tile_pool(name="small", bufs=2))
                ps_sc = esC.enter_context(
                    tc.tile_pool(name="ps_sc", bufs=2, space="PSUM"))
                ps_ctx = esC.enter_context(
                    tc.tile_pool(name="ps_ctx", bufs=2, space="PSUM"))
                ps_bc = esC.enter_context(
                    tc.tile_pool(name="ps_bc", bufs=2, space="PSUM"))

                # prefetch the first three staircases before the wo weights
                stair_tiles = {}

                def stair_fetch(h):
                    st = stairp.tile([128, SW], BF16, tag="stair", name="stair")
                    nc.sync.dma_start(out=st, in_=bass.AP(
                        tensor=rbrev[:, :].tensor,
                        offset=h * RBW, ap=[[1, 128], [1, SW]]))
                    stair_tiles[h] = st

                for h in range(3):
                    stair_fetch(h)

                wo_sb = [wop.tile([128, E], BF16, tag=f"wo{i}", name=f"wo{i}")
                         for i in range(KT)]
                for i in range(KT):
                    nc.sync.dma_start(out=wo_sb[i],
                                      in_=wo_t[i * 128:(i + 1) * 128, :])

                # software pipeline state
                pend_ctx = None      # (jt pair) ctx MMs deferred one pair
                pend_norm = None     # per-head normalize chain, one head late

                def emit_norm(h, ps_c):
                    """sum copy + reciprocal now (DVE); broadcast MM and
                    ctx multiply are returned as a closure emitted later."""
                    kt, half = h // 2, (h % 2) * 64
                    nc.vector.tensor_copy(sums_sb, ps_c[64:65, :])
                    nc.vector.tensor_copy(hctx[h], ps_c[0:64, :])
                    nc.vector.reciprocal_approx_fast(out=recf_sb, in_=sums_sb)
                    nc.vector.tensor_copy(rec_sb[0:1, :], recf_sb)

                    def fin():
                        pr = ps_bc.tile([128, QW], F32, tag="pbc", name="pbc")
                        nc.tensor.matmul(pr, e0_sb, rec_sb,
                                         start=True, stop=True)
                        nc.vector.tensor_tensor(
                            out=ctx_sb[kt][half:half + 64, :],
                            in0=hctx[h], in1=pr[0:64, :], op=ALU.mult)
                    return fin

                for h in range(H):
                    kt, half = h // 2, (h % 2)
                    if h + 2 < H and (h + 2) not in stair_tiles:
                        stair_fetch(h + 2)
                    stair = stair_tiles.pop(h)
                    ps_c = ps_ctx.tile([65, QW], F32, tag="psc", name="psc")
                    for jp in range(4):
                        # gated staircase for the two jt of this pair
                        Gs = []
                        for jj in range(2):
                            jt = 2 * jp + jj
                            ms = 896 - jt * 128
                            G = gp.tile([128, QW], BF16, tag="G", name="G")
                            nc.vector.tensor_tensor(
                                out=G, in0=stair[:, ms:ms + QW],
                                in1=gate_bc[h], op=ALU.mult)
                            Gs.append(G)
                        # two-bank score tile; exp covers both halves
                        ps2 = ps_sc.tile([128, 2 * QW], F32, tag="ps2",
                                         name="ps2")
                        for jj in range(2):
                            jt = 2 * jp + jj
                            o = ps2[:, jj * QW:(jj + 1) * QW]
                            nc.tensor.matmul(
                                o, kT_sb[kt][:, jt * 128:(jt + 1) * 128],
                                qTz_sb[kt][:, half * QW:half * QW + QW],
                                start=True, stop=False)
                            nc.tensor.matmul(o, anti_sb, Gs[jj], start=False,
                                             stop=True)
                        if pend_ctx is not None:
                            pend_ctx()
                            pend_ctx = None
                        if jp == 1 and pend_norm is not None:
                            pend_norm()
                            pend_norm = None
                        expT = expp.tile([128, 2 * QW], BF16, tag="expt",
                                         name="expt")
                        nc.scalar.activation(expT, ps2, AF.Exp)

                        def mk_ctx(jp, expT, ps_c):
                            def emit():
                                for jj in range(2):
                                    jt = 2 * jp + jj
                                    nc.tensor.matmul(
                                        ps_c,
                                        vTok_sb[jt][:, (kt * 2 + half) * 65:
                                                    (kt * 2 + half) * 65 + 65],
                                        expT[:, jj * QW:(jj + 1) * QW],
                                        start=(jt == 0), stop=(jt == TT - 1))
                            return emit
                        pend_ctx = mk_ctx(jp, expT, ps_c)
                    # flush last pair's ctx before the next head's scores
                    pend_ctx()
                    pend_ctx = None
                    pend_norm = emit_norm(h, ps_c)
                pend_norm()
                pend_norm = None

                # ---------------- output projection ----------------
                for i_o in range(KT):
                    c_o = slice(i_o * 128, (i_o + 1) * 128)
                    p = ps_bc.tile([128, QW], F32, tag="pbc", name="pbc")
                    for i in range(KT):
                        nc.tensor.matmul(p, wo_sb[i][:, c_o], ctx_sb[i],
                                         start=(i == 0), stop=(i == KT - 1))
                    o_sb = smallp.tile([128, QW], F32, tag="osb", name="osb")
                    nc.vector.tensor_scalar_add(o_sb, p,
                                                bias_cols["o"][:, i_o:i_o + 1])
                    nc.sync.dma_start(out=outT[c_o, :], in_=o_sb)

    nc.finalize()
    return nc


_NC_CACHE = None


def _get_nc():
    global _NC_CACHE
    if _NC_CACHE is None:
        _NC_CACHE = _build_program()
    return _NC_CACHE


def kernel(hidden_states, Wq, bq, Wk, bk, Wv, bv,
           Aq, Bq, Ak, Bk, Av, Bv, Wo, bo, Wg, bg, gru_const, rel_embed):
    import ml_dtypes

    BF = ml_dtypes.bfloat16
    hidden_states = np.asarray(hidden_states, dtype=np.float32)
    f = lambda a: np.ascontiguousarray(np.asarray(a, dtype=np.float32))

    # ---- fold the double projection (+LoRA) into one affine map ----
    def fold(W, b, A, Bm, scale=1.0):
        W, b, A, Bm = f(W), f(b), f(A), f(Bm)
        M = (W.T + 0.5 * (A.T @ Bm.T)) @ W.T * scale
        be = (b @ W.T + b) * scale
        return M, be

    Mq, bq_e = fold(Wq, bq, Aq, Bq, float(HD) ** -0.5)
    Mk, _ = fold(Wk, bk, Ak, Bk)          # k bias is softmax-invariant
    Mv, bv_e = fold(Wv, bv, Av, Bv)

    wv_a = np.zeros((E, VW), np.float32)
    ones_row = np.zeros(VW, np.float32)
    for h in range(H):
        wv_a[:, h * 65:h * 65 + 64] = Mv[:, h * 64:(h + 1) * 64]
        ones_row[h * 65 + 64] = 1.0
    ones_rep = np.broadcast_to(ones_row, (128, VW))

    Wo_f = f(Wo)
    bo_eff = f(bo) + Wo_f @ bv_e          # bv folded through softmax

    shared = {
        "wq_e": Mq.astype(BF), "wk_e": Mk.astype(BF),
        "wv_a": wv_a.astype(BF), "wo_t": np.ascontiguousarray(Wo_f.T).astype(BF),
        "bq_c": bq_e.reshape(E, 1).astype(np.float32),
        "bo_c": bo_eff.reshape(E, 1).astype(np.float32),
        "ones_rep": np.ascontiguousarray(ones_rep.astype(BF)),
    }
    anti = np.zeros((128, 128), np.float32)
    anti[np.arange(128), 127 - np.arange(128)] = 1.0
    shared["anti"] = anti.astype(BF)
    e0 = np.zeros((128, 128), np.float32)
    e0[0, :] = 1.0
    shared["e0_pad"] = e0
    sel = np.zeros((128, H * 128), np.float32)
    for h in range(H):
        sel[h, h * 128:(h + 1) * 128] = 1.0
    shared["sel_pad"] = sel.astype(BF)
    # gate projection: fold the reshape(2,4).sum(-1) into the weights and lay
    # out block-diagonally per head. gru_const == 1 is folded into the gate
    # algebra (gate = ga*gb - ga + 2).
    Wg_np, bg_np = f(Wg), f(bg)
    wg2 = Wg_np.reshape(2, 4, HD).sum(1)            # [2, HD]
    bg2 = bg_np.reshape(2, 4).sum(1)                # [2]
    wg_big = np.zeros((E, 64), np.float32)
    for h in range(H):
        wg_big[h * HD:(h + 1) * HD, h] = wg2[0]
        wg_big[h * HD:(h + 1) * HD, 32 + h] = wg2[1]
    shared["wg_big"] = wg_big.astype(BF)
    bg_c = np.zeros((64, 1), np.float32)
    bg_c[:H, 0] = bg2[0]
    bg_c[32:32 + H, 0] = bg2[1]
    shared["bg_c"] = bg_c

    # host-computed reversed rb table:
    # rbrev[h, u] = rel_embed[b1d[2046 - th*512 - u], h] (0 where invalid)
    b1d = _bucket1d()
    rel = f(rel_embed)
    rbrev = {}
    for th in range(2):
        m = np.zeros((H, RBW), np.float32)
        u = np.arange(RBW)
        src = 2046 - th * QW - u
        ok = (src >= 0) & (src <= 2046)
        m[:, u[ok]] = rel[b1d[src[ok]], :].T
        rbrev[th] = m.astype(BF)

    xT_all = hidden_states.transpose(0, 2, 1).astype(BF)  # [B,E,T] bf16

    in_maps = []
    for c in range(N_CORES):
        b_, th = c // 2, c % 2
        im = dict(shared)
        im["xT"] = np.ascontiguousarray(xT_all[b_])
        im["xq"] = np.ascontiguousarray(xT_all[b_][:, th * QW:(th + 1) * QW])
        im["rbrev"] = rbrev[th]
        in_maps.append(im)

    nc = _get_nc()
    res = run_bass_kernel_spmd(nc, in_maps, core_ids=list(range(N_CORES)))

    out = np.empty((B, T, E), np.float32)
    for c in range(N_CORES):
        b_, th = c // 2, c % 2
        out[b_, th * QW:(th + 1) * QW, :] = res.results[c]["outT"].T
    return out
